# revision 43
# baseline (speedup 1.0000x reference)
"""Distributed Trainium2 kernel for a causal multi-head self-attention block.

  out = softmax_causal((x@Wq+bq)(x@Wk+bk)^T / sqrt(Dh)) (x@Wv+bv) @ W_out + b_out

Sharding (8 NeuronCores, tensor-parallel over heads):
  - Each core owns 2 of the 16 heads, both batches -> 4 (batch, head) units.
  - Host packs x / weights partition-contiguous so every DMA streams large
    linear reads; all loads are split across both HW-DGE rings in priority
    order (wq, x-chunk0, wk, wv, x-chunk1..3, W_out, x-batch1).
  - The emission is a single fine-grained weave: attention tile-work is
    queued as its inputs are produced and paced into the QKV/projection
    matmul stream under a scalar-engine deficit cap, so the exp work (the
    attention bottleneck) hides under the big matmuls and the PE never
    waits on the activation engine.
  - Attention: scoresT = kT-tile.T @ qT (t on partitions, s free); exp with
    no max subtraction (scores ~ N(0,1), safe); causal at tile granularity
    with a triangular mask multiply on diagonal tiles; PV matmul uses
    [v | ones] so the softmax denominator falls out of PSUM column 128.
  - Chunk tails: normalize on DVE, then PE-transpose to [dh, s] one chunk
    later (so the PE never waits on the vector engine); the v-bias is
    folded into b_out on the host (softmax weights sum to 1, so it
    commutes), removing the post-transpose bias adds. Two AllToAlls per
    batch (one per owned head) redistribute head-shards -> token-shards.
  - Output projection is token-parallel with the full W_out. proj(b0) and
    a split even/odd-head proj(b1) (paired accumulators packed two output
    chunks per PSUM bank) overlap the b1 collectives' flight. Host
    reassembles, transposes, and adds the effective bias.

All matmul operands are bf16 (1 cycle/row on the PE), accumulation f32.
"""

import math
import numpy as np
import ml_dtypes

import sys

for _p in ("/opt/trn_rl_repo",):
    if _p not in sys.path:
        sys.path.insert(0, _p)

import concourse.bass as bass
import concourse.bacc as bacc
import concourse.mybir as mybir
import concourse.tile as tile
from concourse.bass_utils import run_bass_kernel_spmd

BF16 = mybir.dt.bfloat16
F32 = mybir.dt.float32
NPBF16 = ml_dtypes.bfloat16

B, S, D = 2, 2048, 2048
H, DH = 16, 128
NC = 8
HL = H // NC            # heads per core = 2
SC = 512                # s-chunk (free dim of scores matmul)
NCH = S // SC           # 4 s-chunks per batch
NT = S // 128           # 16 t-tiles per batch
TOKB = S // NC          # 256 tokens owned per core per batch
INV_SQRT_DH = 1.0 / math.sqrt(DH)

# emission pacing: keep the scalar engine's outstanding exp work under this
# many ns so in-order PE stalls on score-psum recycling stay tiny
DEFICIT_CAP = 2000.0


def build_kernel():
    nc = bacc.Bacc("TRN2", target_bir_lowering=False, debug=False, num_devices=NC)

    # host-packed, partition-contiguous layouts
    xp = nc.declare_dram_parameter("xp", [B, NCH, 128, 16, SC], BF16, isOutput=False)
    wq = nc.declare_dram_parameter("wq", [128, HL, 16, 128], BF16, isOutput=False)
    wk = nc.declare_dram_parameter("wk", [128, HL, 16, 128], BF16, isOutput=False)
    wv = nc.declare_dram_parameter("wv", [128, 16, HL * 128], BF16, isOutput=False)
    bq = nc.declare_dram_parameter("bq", [128, HL, 1], F32, isOutput=False)
    bk = nc.declare_dram_parameter("bk", [128, HL, 1], F32, isOutput=False)
    w_out = nc.declare_dram_parameter("w_out", [128, 16, D], BF16, isOutput=False)
    ident = nc.declare_dram_parameter("ident", [128, 128], BF16, isOutput=False)
    maskp = nc.declare_dram_parameter("maskp", [128, 128], BF16, isOutput=False)
    out = nc.declare_dram_parameter("out", [B, D, TOKB], BF16, isOutput=True)

    with tile.TileContext(nc) as tc:
        with (
            tc.tile_pool(name="wpool", bufs=1) as wpool,
            tc.tile_pool(name="xpool", bufs=2) as xpool,
            tc.tile_pool(name="qkv", bufs=2) as qkvpool,
            tc.tile_pool(name="expp", bufs=4) as expp,
            tc.tile_pool(name="small", bufs=4) as small,
            tc.tile_pool(name="wo", bufs=1) as wopool,
            tc.tile_pool(name="rcv", bufs=2) as rcvpool,
            tc.tile_pool(name="outp", bufs=2) as outp,
            tc.tile_pool(name="psum", bufs=2, space="PSUM") as psum,
            tc.tile_pool(name="dram", bufs=1, space="DRAM") as dram,
        ):
            # ---- startup-priority loads, split across both HW-DGE rings ----
            wq_t = wpool.tile([128, HL, 16, 128], BF16, tag="wq")
            wk_t = wpool.tile([128, HL, 16, 128], BF16, tag="wk")
            wv_t = wpool.tile([128, 16, HL * 128], BF16, tag="wv")
            bq_t = wpool.tile([128, HL, 1], F32, tag="bq")
            bk_t = wpool.tile([128, HL, 1], F32, tag="bk")
            id_t = wpool.tile([128, 128], BF16, tag="ident")
            mask_t = wpool.tile([128, 128], BF16, tag="maskp")

            def split_load(dst, src, mid):
                nc.sync.dma_start(dst[:, :mid], src[:, :mid])
                nc.scalar.dma_start(dst[:, mid:], src[:, mid:])

            def load_x_chunk(b, c, pieces=1):
                # pieces>1: d-sliced piece loads so the first accumulation
                # groups can start before the whole chunk lands (deps are
                # AP-range-based)
                xt = xpool.tile([128, 16, SC], BF16, tag="xt",
                                name=f"xt_{b}_{c}")
                w = 8 // pieces
                for p in range(pieces):
                    nc.sync.dma_start(xt[:, p * w : (p + 1) * w],
                                      xp[b, c, :, p * w : (p + 1) * w])
                    nc.scalar.dma_start(xt[:, 8 + p * w : 8 + (p + 1) * w],
                                        xp[b, c, :, 8 + p * w : 8 + (p + 1) * w])
                return xt

            # startup: 3-way split across sync, scalar AND the gpsimd SW-DGE
            # ring so the critical mass (wq + chunk0 + wk + wv) lands sooner
            nc.gpsimd.dma_start(bq_t[:], bq[:])
            nc.gpsimd.dma_start(bk_t[:], bk[:])
            nc.gpsimd.dma_start(id_t[:], ident[:])
            nc.gpsimd.dma_start(mask_t[:], maskp[:])
            split_load(wq_t, wq[:], 1)
            xt00 = xpool.tile([128, 16, SC], BF16, tag="xt", name="xt_0_0")
            for p in range(3):
                nc.sync.dma_start(xt00[:, 2 * p : 2 * p + 2],
                                  xp[0, 0, :, 2 * p : 2 * p + 2])
                nc.scalar.dma_start(xt00[:, 6 + 2 * p : 8 + 2 * p],
                                    xp[0, 0, :, 6 + 2 * p : 8 + 2 * p])
            nc.gpsimd.dma_start(xt00[:, 12:16], xp[0, 0, :, 12:16])
            xts = {(0, 0): xt00}
            split_load(wk_t, wk[:], 1)
            split_load(wv_t, wv[:], 8)
            xts[(0, 1)] = load_x_chunk(0, 1)
            wo_t = wopool.tile([128, 16, D], BF16, tag="wo")

            # per-(batch, head) A2A bounce buffers (DRAM). Layout
            # [dest, dh, st, s] so the receive load is a straight copy with
            # 512B-contiguous lines per partition.
            a2a_in = [
                [dram.tile([NC, 128, 2, 128], BF16, tag=f"a2a_in{b}_{hl}",
                           name=f"a2a_in{b}_{hl}") for hl in range(HL)]
                for b in range(B)
            ]
            a2a_out = [
                [dram.tile([NC, 128, 2, 128], BF16, tag=f"a2a_out{b}_{hl}",
                           name=f"a2a_out{b}_{hl}") for hl in range(HL)]
                for b in range(B)
            ]

            # ---------- fine-grained weave machinery ----------
            from collections import deque
            attn_q = deque()          # (pe_ns, sc_ns, closure)
            state = {"deficit": 0.0}

            def pump(pe_ns):
                """Called after emitting pe_ns of big-matmul work: drain
                queued attention work while the scalar engine stays ahead."""
                state["deficit"] = max(0.0, state["deficit"] - pe_ns)
                while attn_q and state["deficit"] < DEFICIT_CAP:
                    pe, sc, fn = attn_q.popleft()
                    fn()
                    state["deficit"] = max(0.0, state["deficit"] + sc - pe)

            def drain():
                while attn_q:
                    pe, sc, fn = attn_q.popleft()
                    fn()
                state["deficit"] = 0.0

            def qkv_chunk(b, scn, xt, qTb, kTb, vvb):
                for w_t, b_t, dst in ((wq_t, bq_t, qTb), (wk_t, bk_t, kTb)):
                    for hl in range(HL):
                        ps = psum.tile([128, SC], F32, tag="mm",
                                       name=f"psqk_{b}_{scn}_{hl}_{id(dst)}")
                        for d in range(16):
                            nc.tensor.matmul(
                                ps[:], w_t[:, hl, d], xt[:, d],
                                start=(d == 0), stop=(d == 15),
                            )
                        nc.vector.tensor_scalar_add(
                            dst[:, hl, scn * SC : (scn + 1) * SC],
                            ps[:], b_t[:, hl],
                        )
                        pump(4096.0)
                for ts in range(SC // 128):
                    ps = psum.tile([128, HL * 128], F32, tag="mm",
                                   name=f"psv_{b}_{scn}_{ts}")
                    for d in range(16):
                        nc.tensor.matmul(
                            ps[:],
                            xt[:, d, ts * 128 : (ts + 1) * 128],
                            wv_t[:, d],
                            start=(d == 0), stop=(d == 15),
                        )
                    tt_idx = scn * (SC // 128) + ts
                    for hl in range(HL):
                        nc.vector.tensor_copy(
                            vvb[:, hl, tt_idx, 0:128],
                            ps[:, hl * 128 : (hl + 1) * 128],
                        )
                    pump(2048.0)

            pending_xpose = {}

            def queue_attn_chunk(b, hl, scn, qTb, kTb, vvb):
                holder = {}

                def mk_tt(tt):
                    off = max(0, tt - 4 * scn)  # first live s-subtile
                    nlive = 4 - off
                    s0 = scn * SC + off * 128

                    def fn():
                        if "o2" not in holder:
                            holder["o2"] = [
                                psum.tile([128, 2, 129], F32, tag="o2", bufs=2,
                                          name=f"o2_{b}_{hl}_{scn}_{i}")
                                for i in range(2)
                            ]
                        o2 = holder["o2"]
                        sp = psum.tile([128, SC], F32, tag="sp", bufs=4,
                                       name=f"sp_{b}_{hl}_{scn}_{tt}")
                        nc.tensor.matmul(
                            sp[:, : nlive * 128],
                            kTb[:, hl, tt * 128 : (tt + 1) * 128],
                            qTb[:, hl, s0 : (scn + 1) * SC],
                            start=True, stop=True,
                        )
                        ex = expp.tile([128, SC], BF16, tag="ex",
                                       name=f"ex_{b}_{hl}_{scn}_{tt}")
                        nc.scalar.activation(
                            ex[:, : nlive * 128], sp[:, : nlive * 128],
                            mybir.ActivationFunctionType.Exp,
                            scale=INV_SQRT_DH,
                        )
                        if tt >= 4 * scn:  # diagonal sub-block: causal mask
                            nc.vector.tensor_mul(
                                ex[:, 0:128], ex[:, 0:128], mask_t[:]
                            )
                        for ss in range(off, 4):
                            st = 4 * scn + ss
                            # start=True clears has_written BANK-wide; only
                            # the first matmul touching each o2 bank may set
                            # it (see v|ones trick).
                            nc.tensor.matmul(
                                o2[ss // 2][:, ss % 2, :],
                                ex[:, (ss - off) * 128 : (ss - off + 1) * 128],
                                vvb[:, hl, tt],
                                start=(tt == 0 and ss % 2 == 0),
                                stop=(tt == st),
                            )

                    pe = nlive * 128.5 + 80.0
                    sc = nlive * 106.7 + 350.0
                    return (pe, sc, fn)

                def normalize():
                    o2 = holder["o2"]
                    ans = []
                    for ss in range(4):
                        o2t = o2[ss // 2]
                        rc = small.tile([128, 1], F32, tag="rc",
                                        name=f"rc_{b}_{hl}_{scn}_{ss}")
                        nc.vector.reciprocal(rc[:], o2t[:, ss % 2, 128:129])
                        an = small.tile([128, 128], BF16, tag="an", bufs=8,
                                        name=f"an_{b}_{hl}_{scn}_{ss}")
                        nc.vector.tensor_scalar_mul(
                            an[:], o2t[:, ss % 2, 0:128], rc[:]
                        )
                        ans.append(an)
                    holder["ans"] = ans

                def xpose():
                    # PE transposes of the previous chunk's normalized
                    # output: deferred one chunk so the DVE normalize is
                    # long done and the PE never waits on it.
                    ans = holder["ans"]
                    for ss in range(4):
                        st = 4 * scn + ss
                        tp = psum.tile([128, 128], BF16, tag="sp", bufs=4,
                                       name=f"tp_{b}_{hl}_{scn}_{ss}")
                        nc.tensor.transpose(tp[:], ans[ss][:], id_t[:])
                        att = small.tile([128, 128], BF16, tag="att",
                                         name=f"att_{b}_{hl}_{scn}_{ss}")
                        nc.vector.tensor_copy(att[:], tp[:])
                        nc.gpsimd.dma_start(
                            a2a_in[b][hl][st // 2, :, st % 2], att[:]
                        )
                    if scn == NCH - 1:
                        nc.gpsimd.collective_compute(
                            "AllToAll",
                            mybir.AluOpType.bypass,
                            ins=[a2a_in[b][hl].opt()],
                            outs=[a2a_out[b][hl].opt()],
                            replica_groups=[list(range(NC))],
                        )
                        if hl == HL - 1:
                            state[f"cc{b}_done"] = True

                prev = pending_xpose.pop(hl, None)
                if prev is not None:
                    attn_q.append((450.0, 0.0, prev))
                for tt in range(4 * scn + 4):
                    attn_q.append(mk_tt(tt))
                attn_q.append((0.0, 0.0, normalize))
                if scn == NCH - 1:
                    attn_q.append((450.0, 0.0, xpose))
                else:
                    pending_xpose[hl] = xpose

            def load_rcv(b, rcv, shls=(0, 1), eng=None):
                # On the sync ring: these triggers wait on the collective, and
                # nothing urgent sits behind them there (the scalar ring would
                # head-of-line-block the next batch's exp work).
                for dt in range(16):
                    srcc, shl = dt // HL, dt % HL
                    if shl not in shls:
                        continue
                    (eng or nc.sync).dma_start(rcv[:, dt],
                                               a2a_out[b][shl][srcc])

            def proj_phase(b, rcv, ocs=range(16)):
                for oc in ocs:
                    ps = psum.tile([128, TOKB], F32, tag="mm",
                                   name=f"pso_{b}_{oc}")
                    for dt in range(16):
                        nc.tensor.matmul(
                            ps[:],
                            wo_t[:, dt, oc * 128 : (oc + 1) * 128],
                            rcv[:, dt],
                            start=(dt == 0), stop=(dt == 15),
                        )
                    ot = outp.tile([128, TOKB], BF16, tag="ot", bufs=4,
                                   name=f"ot_{b}_{oc}")
                    nc.vector.tensor_copy(ot[:], ps[:])
                    nc.sync.dma_start(out[b, oc * 128 : (oc + 1) * 128, :], ot[:])
                    pump(2048.0)

            # ---------------- program order ----------------
            rcv_t = [None, None]
            flat = [(b, c) for b in range(B) for c in range(NCH)]
            for b in range(B):
                qTb = qkvpool.tile([128, HL, S], BF16, tag="qT", name=f"qT{b}")
                kTb = qkvpool.tile([128, HL, S], BF16, tag="kT", name=f"kT{b}")
                vvb = qkvpool.tile([128, HL, NT, 129], BF16, tag="vv",
                                   name=f"vv{b}")
                nc.gpsimd.memset(vvb[:, :, :, 128:129], 1.0)
                for scn in range(NCH):
                    qkv_chunk(b, scn, xts[(b, scn)], qTb, kTb, vvb)
                    # prefetch the x chunk two steps ahead: the WAR dep on
                    # the chunk this slot replaces is correct here, and the
                    # ring position keeps priority order.
                    fi = flat.index((b, scn)) + 2
                    if fi < len(flat):
                        xts[flat[fi]] = load_x_chunk(*flat[fi])
                    if b == 0 and scn == 1:
                        # W_out behind the startup-critical stream
                        split_load(wo_t, w_out[:], 8)
                    for hl in range(HL):
                        queue_attn_chunk(b, hl, scn, qTb, kTb, vvb)
                    if b == 1 and scn == 1:
                        # rcv0 after b1's x loads on the sync ring (its
                        # triggers block on the b0 collectives); force-drain
                        # until those collectives are emitted.
                        while not state.get("cc0_done"):
                            _, _, fn = attn_q.popleft()
                            fn()
                        rcv_t[0] = rcvpool.tile([128, 16, TOKB], BF16,
                                                tag="rcv", name="rcv0")
                        load_rcv(0, rcv_t[0])

            # proj(0) first half paces the remaining b1 attention backlog;
            # the rest is interleaved below so the PE never idles while b1's
            # collectives fly.
            proj_phase(0, rcv_t[0], range(8))
            drain()
            proj_phase(0, rcv_t[0], range(8, 12))
            rcv_t[1] = rcvpool.tile([128, 16, TOKB], BF16, tag="rcv",
                                    name="rcv1")
            # proj(1) in two passes: even-dt (head-pair 0) contributions
            # arrive with b1's FIRST collective, so their accumulation runs
            # during the second collective's flight; odd dts finish after it
            # lands. Six paired accumulators [128,2,TOKB] live on the sp/o2
            # tags only, leaving the mm tag free for the proj(0) remainder
            # to bridge the evens->odds handover.
            load_rcv(1, rcv_t[1], shls=(0,), eng=nc.scalar)
            tags = ["sp", "sp", "sp", "sp", "o2", "o2"]
            tag_bufs = {"sp": 4, "o2": 2}
            pps = [
                psum.tile([128, 2, TOKB], F32, tag=tags[p],
                          bufs=tag_bufs[tags[p]], name=f"pp1_{p}")
                for p in range(6)
            ]
            # dt-major waves: each wave needs only the rcv piece that just
            # arrived, so the PE runs right behind the split rcv loads
            for j, dt in enumerate(range(0, 16, 2)):
                for p in range(6):
                    for och in range(2):
                        oc = 2 * p + och
                        # start=True clears has_written bank-wide: only the
                        # tile's very first matmul may set it
                        nc.tensor.matmul(
                            pps[p][:, och],
                            wo_t[:, dt, oc * 128 : (oc + 1) * 128],
                            rcv_t[1][:, dt],
                            start=(och == 0 and j == 0), stop=False,
                        )
            load_rcv(1, rcv_t[1], shls=(1,), eng=nc.scalar)
            # bridge: the last proj(0) chunks run while the second collective
            # finishes its flight
            proj_phase(0, rcv_t[0], range(12, 16))
            # odds: dt-major for the first waves (runs right behind the rcv
            # loads), then oc-major for the rest so the output copies and
            # stores pipeline with the remaining matmuls
            for dt in (1, 3, 5, 7):
                for p in range(6):
                    for och in range(2):
                        oc = 2 * p + och
                        nc.tensor.matmul(
                            pps[p][:, och],
                            wo_t[:, dt, oc * 128 : (oc + 1) * 128],
                            rcv_t[1][:, dt],
                            start=False, stop=False,
                        )
            for p in range(6):
                for och in range(2):
                    oc = 2 * p + och
                    for dt in (9, 11, 13, 15):
                        nc.tensor.matmul(
                            pps[p][:, och],
                            wo_t[:, dt, oc * 128 : (oc + 1) * 128],
                            rcv_t[1][:, dt],
                            start=False, stop=(dt == 15),
                        )
                    ot = outp.tile([128, TOKB], BF16, tag="ot", bufs=4,
                                   name=f"ot1_{oc}")
                    nc.vector.tensor_copy(ot[:], pps[p][:, och])
                    nc.sync.dma_start(out[1, oc * 128 : (oc + 1) * 128, :],
                                      ot[:])
            # final four output chunks: both head-pairs are in SBUF by now
            for oc in range(12, 16):
                ps = psum.tile([128, TOKB], F32, tag="mm",
                               name=f"pso1_{oc}")
                for dt in range(16):
                    nc.tensor.matmul(
                        ps[:],
                        wo_t[:, dt, oc * 128 : (oc + 1) * 128],
                        rcv_t[1][:, dt],
                        start=(dt == 0), stop=(dt == 15),
                    )
                ot = outp.tile([128, TOKB], BF16, tag="ot", bufs=4,
                               name=f"ot1f_{oc}")
                nc.vector.tensor_copy(ot[:], ps[:])
                nc.sync.dma_start(out[1, oc * 128 : (oc + 1) * 128, :],
                                  ot[:])

    nc.compile()
    return nc


def make_in_maps(x, W_in, b_in, W_out, b_out):
    # x packed so each SBUF partition reads 16KB contiguous per chunk:
    # xp[b, c, p, d, s] = x[b, c*SC+s, d*128+p]
    xp = np.ascontiguousarray(
        x.reshape(B, NCH, SC, 16, 128).transpose(0, 1, 4, 3, 2)
    ).astype(NPBF16)
    ident = np.eye(128, dtype=NPBF16)
    maskp = np.triu(np.ones((128, 128), dtype=np.float32)).astype(NPBF16)
    # w_out[p, d, m] = W_out[d*128+p, m] (32KB contiguous per partition)
    w_out_t = np.ascontiguousarray(
        W_out.reshape(16, 128, D).transpose(1, 0, 2)
    ).astype(NPBF16)

    in_maps = []
    for c in range(NC):
        hs = [2 * c + hl for hl in range(HL)]  # global head ids
        wq_c = np.ascontiguousarray(
            np.stack(
                [W_in[:, h * 128 : (h + 1) * 128].reshape(16, 128, 128) for h in hs]
            ).transpose(2, 0, 1, 3)
        ).astype(NPBF16)
        wk_c = np.ascontiguousarray(
            np.stack(
                [
                    W_in[:, D + h * 128 : D + (h + 1) * 128].reshape(16, 128, 128)
                    for h in hs
                ]
            ).transpose(2, 0, 1, 3)
        ).astype(NPBF16)
        wv_c = np.ascontiguousarray(
            np.concatenate(
                [
                    W_in[:, 2 * D + h * 128 : 2 * D + (h + 1) * 128].reshape(
                        16, 128, 128
                    )
                    for h in hs
                ],
                axis=2,
            ).transpose(1, 0, 2)
        ).astype(NPBF16)
        bq_c = np.ascontiguousarray(
            np.stack([b_in[h * 128 : (h + 1) * 128] for h in hs], axis=1)
        ).reshape(128, HL, 1).astype(np.float32)
        bk_c = np.ascontiguousarray(
            np.stack([b_in[D + h * 128 : D + (h + 1) * 128] for h in hs], axis=1)
        ).reshape(128, HL, 1).astype(np.float32)
        in_maps.append(
            {
                "xp": xp,
                "wq": wq_c,
                "wk": wk_c,
                "wv": wv_c,
                "bq": bq_c,
                "bk": bk_c,
                "w_out": w_out_t,
                "ident": ident,
                "maskp": maskp,
            }
        )
    return in_maps


_NC_CACHE = {}


def _get_nc():
    if "nc" not in _NC_CACHE:
        _NC_CACHE["nc"] = build_kernel()
    return _NC_CACHE["nc"]


def kernel(x, W_in, b_in, W_out, b_out, _trace=False, **kw):
    x = np.asarray(x, dtype=np.float32)
    W_in = np.asarray(W_in, dtype=np.float32)
    b_in = np.asarray(b_in, dtype=np.float32)
    W_out = np.asarray(W_out, dtype=np.float32)
    b_out = np.asarray(b_out, dtype=np.float32)

    nc = _get_nc()
    in_maps = make_in_maps(x, W_in, b_in, W_out, b_out)
    res = run_bass_kernel_spmd(nc, in_maps, core_ids=list(range(NC)), trace=_trace)
    outf = np.empty((B, S, D), dtype=np.float32)
    for c in range(NC):
        o = np.asarray(res.results[c]["out"])  # [B, D, TOKB]
        for b in range(B):
            outf[b, c * TOKB : (c + 1) * TOKB, :] = o[b].T
    # v-bias commutes through softmax (weights sum to 1): fold into b_out
    b_eff = b_out + b_in[2 * D : 3 * D].astype(np.float64) @ W_out.astype(np.float64)
    outf += b_eff[None, None, :].astype(np.float32)
    if _trace:
        return outf, res
    return outf
